# revision 63
# baseline (speedup 1.0000x reference)
"""Ewald reciprocal-space potential on 8 Trainium2 NeuronCores.

Math: pot[f] = sum_k w(k) |S_f(k)|^2 - sum_n q_n^2 / (sigma (2pi)^1.5),
S_f(k) = sum_n q_n e^{2 pi i (a x + b y + c z)/box}, k = (a,b,c),
a in [0,28], b,c in [-28,28], masked to 0 < |k|^2 <= (2pi/dl)^2.

Device algorithm per core (SPMD, core = 2*frame + col_half):
  per 128-atom tile (broadcast-DMA chunks bring coords in as [87, cols]):
    1. Pool: t = coord * (k/box); fr = t - i32round(t) in [-0.5, 0.5]
    2. PE: theta[atom, col] = fr^T @ twohot (two-hot +/-1 selector cols)
       into PSUM, chunks [0,512)+[512,658) (even widths: fp32r ISA)
    3. DVE: sin block y = add_range_wrap(theta) in [-0.5, 0.5] (GPSIMD
       cannot touch PSUM); cos block = wrap(y + 0.25): first XD cols via a
       second DVE add_range_wrap, the rest on Pool (SBUF source) via
       m = (y > 0.25) - 0.25; yc = y - m
    4. ACT: one Sin(2 pi w) per TILE PAIR over 2*1316 cols -> P
       (pairing halves the per-instruction latency overhead)
    5. Pool: qEz[atom, 128] = q * (cos_z | sin_z) mirrored to c in [-28,28]
    6. PE: S += qEz^T @ P in PSUM over all 32 tiles, 4 chunks of 372/256
       cols placed so every chunk is >=256 cols inside one PSUM bank
  epilogue: S_re/S_im recombine (ACT Identity + DVE, Pool scales by
  sqrt(w)), two Square+accum -> res [128, 4] row partials
  (col0+col1 = w|S|^2 partials, col2 = q^2 partials).
Host: pot[f] = sum(resA[:, 0:2]) + sum(resB[:, 0:2])
               - sum(resA[:, 2])/(sigma (2pi)^1.5).
"""
import numpy as np

# ---- problem constants (hardcoded per harness contract) ----
B = 4
N = 4000
NPAD = 4096
NTILES = NPAD // 128
NK = 28
A = NK + 1            # 29 kx values 0..28
C = 2 * NK + 1        # 57 ky/kz values -28..28
DL = 2.0
SIGMA = 1.0
TWOPI = 2.0 * np.pi
K_SQ_MAX = (TWOPI / DL) ** 2
SELF_CONST = 1.0 / (SIGMA * TWOPI ** 1.5)

# ---- k-column layout (box-independent) ----
_cols_all = [(a, b) for a in range(A) for b in range(-NK, NK + 1)
             if a * a + b * b <= NK * NK]
_half = (len(_cols_all) + 1) // 2
_COL_HALVES = [_cols_all[:_half], _cols_all[_half:]]
E_XY = _half + (_half % 2)  # 628 per half
EH = E_XY + A + 1     # theta cols: xy then z + 1 pad = 658
E2 = 2 * EH           # P layout: [sin(EH) | cos(EH)] = 1316
XD = 280              # cos cols wrapped on DVE (rest on Pool's 2-op path)
SINZ0 = E_XY          # start of sin-z cols (628)
COS0 = EH             # start of cos block (658)
COSZ0 = COS0 + E_XY   # start of cos-z cols (1286)

# S PSUM placement: blocks offset so all 4 matmul chunks are >=256 cols and
# bank-contained.  sin block at [140,768) (chunks 372+256), cos block at
# [1164,1792) (chunks 372+256).
S_SIN = 140
S_COS = 1164
S_CUT = 372           # first chunk length within each block

_compiled = None      # (nc, names) cache within one process


_tables_cache = {}


def _build_tables(h, box):
    """Per-core constant tables for one frame's box and one column half."""
    bx, by, bz = float(box[0]), float(box[1]), float(box[2])
    key = (h, bx, by, bz)
    if key in _tables_cache:
        return _tables_cache[key]
    cols = _COL_HALVES[h]
    # avec: multipliers for the frac tables, rows 0-28 a/bx, 29-57 b/by,
    # 58-86 c/bz
    avec = np.zeros((87, 1), np.float32)
    avec[0:29, 0] = np.arange(29, dtype=np.float32) / np.float32(bx)
    avec[29:58, 0] = np.arange(29, dtype=np.float32) / np.float32(by)
    avec[58:87, 0] = np.arange(29, dtype=np.float32) / np.float32(bz)

    twohot = np.zeros((96, EH), np.float32)
    for j, (a, b) in enumerate(cols):
        twohot[a, j] = 1.0
        if b != 0:
            twohot[29 + abs(b), j] = np.sign(b)
    for c in range(A):
        twohot[58 + c, SINZ0 + c] = 1.0

    # sqrt(w) table [57 c-rows, E_XY cols], fp32 arithmetic mirroring
    # reference
    cs = np.arange(-NK, NK + 1, dtype=np.float32)
    sq_c = np.square(cs / np.float32(bz))
    w = np.zeros((C, E_XY), np.float32)
    vol = np.float32(bx) * np.float32(by) * np.float32(bz)
    for j, (a, b) in enumerate(cols):
        ab = (np.square(np.float32(a) / np.float32(bx))
              + np.square(np.float32(b) / np.float32(by)))
        k_sq = np.float32(TWOPI ** 2) * (np.float32(ab) + sq_c)  # (57,)
        mask = (k_sq <= np.float32(K_SQ_MAX)) & (k_sq > 0)
        k_sq_safe = np.where(k_sq > 0, k_sq, np.float32(1.0))
        kfac = np.where(
            mask,
            np.exp(np.float32(-SIGMA ** 2 / 2.0) * k_sq) / k_sq_safe,
            np.float32(0.0))
        factor = np.float32(1.0 if a == 0 else 2.0)
        w[:, j] = kfac * factor / vol
    out = (avec, twohot, np.sqrt(w).astype(np.float32))
    _tables_cache[key] = out
    return out


def _build_module():
    import concourse.bacc as bacc
    import concourse.mybir as mybir
    import concourse.tile as tile

    f32 = mybir.dt.float32
    f32r = mybir.dt.float32r
    i32 = mybir.dt.int32
    AF = mybir.ActivationFunctionType
    OP = mybir.AluOpType

    nc = bacc.Bacc("TRN2")
    posT = nc.dram_tensor("posT", [3, NPAD], f32, kind="ExternalInput")
    qcol = nc.dram_tensor("qcol", [NPAD, 1], f32, kind="ExternalInput")
    avec = nc.dram_tensor("avec", [87, 1], f32, kind="ExternalInput")
    twohot = nc.dram_tensor("twohot", [96, EH], f32, kind="ExternalInput")
    sqw = nc.dram_tensor("sqw", [C, E_XY], f32, kind="ExternalInput")
    res = nc.dram_tensor("res", [128, 4], f32, kind="ExternalOutput")

    # coord broadcast-DMA chunk boundaries (cols); small first chunks so
    # tile 0's coords arrive fast, 512-col (4-tile) chunks afterwards
    CB = [0, 256, 512, 1024, 1536, 2048, 2560, 3072, 3584, 4096]

    with tile.TileContext(nc) as tc:
        with tc.tile_pool(name="const", bufs=1) as cpool, \
             tc.tile_pool(name="bt", bufs=3) as btpool, \
             tc.tile_pool(name="work", bufs=3) as wpool, \
             tc.tile_pool(name="qez", bufs=2) as qpool, \
             tc.tile_pool(name="epi", bufs=1) as epool, \
             tc.tile_pool(name="theta", bufs=2, space="PSUM") as tpsum, \
             tc.tile_pool(name="acc", bufs=1, space="PSUM") as apsum:

            # constants on Pool's SWDGE queue so the SP queue only carries
            # coord chunks; avec and twohot first (tile 0's critical path)
            th_sb = cpool.tile([96, EH], f32)
            nc.gpsimd.dma_start(th_sb[:], twohot[:])
            av_sb = cpool.tile([87, 1], f32)
            nc.gpsimd.dma_start(av_sb[:], avec[:])
            th_r = cpool.tile([96, EH], f32r)
            nc.vector.tensor_copy(th_r[:], th_sb[:])
            qall_sb = cpool.tile([128, NTILES], f32)
            nc.gpsimd.dma_start(
                qall_sb[:], qcol.rearrange("(o p) one -> p (o one)", p=128))

            frs = []
            for j in range(2):
                frt = cpool.tile([96, 128], f32r, tag=f"fr{j}")
                nc.gpsimd.memset(frt[64:96, :].bitcast(f32), 0.0)
                frs.append(frt)

            acc = cpool.tile([128, 4], f32)
            nc.vector.memset(acc[:], 0.0)
            npi_sb = cpool.tile([128, 1], f32)
            nc.vector.memset(npi_sb[:], float(-np.pi))
            scratch = cpool.tile([128, 1], f32)

            # S accumulators in separate PSUM tiles (epilogue readers of
            # one block need not wait on the other): rows = qEz cols
            # (Re 0-56, Im 64-120); each block at cols [140,768)
            Ssin = apsum.tile([128, 1024], f32, tag="Ssin")
            Scos = apsum.tile([128, 1024], f32, tag="Scos")

            # dummy Sin up front: forces the trig act-table load (which also
            # holds Square) off the first real Sin's critical path; the
            # dummy matmul starts the PE p-state ramp clock early
            nc.scalar.activation(scratch[:], npi_sb[:], AF.Sin)
            nc.tensor.matmul(Scos[0:1, 800:801], npi_sb[:], npi_sb[:, 0:1],
                             start=True, stop=True, skip_group_check=True)

            chunk = -1
            for i in range(NTILES):
                if 128 * i >= CB[chunk + 1]:
                    chunk += 1
                    c0, csz = CB[chunk], CB[chunk + 1] - CB[chunk]
                    Bt = btpool.tile([87, 512], f32, tag="Bt")
                    for g in range(3):
                        # chunk 0 is on tile 0's critical path: spread its
                        # three broadcast groups over SP + ACT queues
                        eng = nc.scalar if (chunk == 0 and g == 1) else nc.sync
                        eng.dma_start(
                            Bt[29 * g:29 * (g + 1), 0:csz],
                            posT[g:g + 1, c0:c0 + csz]
                            .to_broadcast([29, csz]))
                b0 = 128 * i - c0

                # fr = t - round(t), t = coord * (k/box), in [-0.5, 0.5]
                fr = frs[i % 2]
                t = wpool.tile([87, 128], f32, tag="t")
                nc.gpsimd.tensor_scalar(t[:], Bt[:, b0:b0 + 128],
                                        av_sb[:], None, OP.mult)
                ti = wpool.tile([87, 128], i32, tag="ti")
                nc.gpsimd.tensor_copy(ti[:], t[:])
                nc.gpsimd.tensor_tensor(fr[0:87, :], t[:], ti[:],
                                        OP.subtract)

                # theta in one PSUM tile, chunks [0,512)+[512,658)
                # (even widths: fp32r ISA requirement)
                th = tpsum.tile([128, 1024], f32, tag="th")
                nc.tensor.matmul(th[:, 0:512], fr[:], th_r[:, 0:512],
                                 start=True, stop=True)
                nc.tensor.matmul(th[:, 512:EH], fr[:], th_r[:, 512:EH],
                                 start=True, stop=True)

                # sin block: one DVE add_range_wrap (GPSIMD cannot touch
                # PSUM).  cos block = wrap(theta + 0.25): first XD cols on
                # DVE, the rest on Pool via m = (y > 0.25) - 0.25; yc = y - m
                single = i >= NTILES - 2
                lead = (i % 2 == 0) or single
                if lead:
                    w2 = wpool.tile([128, 2 * E2], f32, tag="w2")
                    P2 = wpool.tile([128, 2 * E2], f32r, tag="P2")
                woff = 0 if lead else E2
                w_sb = w2[:, woff:woff + E2]
                nc.vector.add_range_wrap(w_sb[:, 0:EH], th[:, 0:EH],
                                         shift=0.0, bound=0.5, period=1.0)
                nc.vector.add_range_wrap(w_sb[:, COS0:COS0 + XD],
                                         w_sb[:, 0:XD],
                                         shift=0.25, bound=0.5, period=1.0)
                mk = qpool.tile([128, EH - XD], f32, tag="mk")
                nc.gpsimd.tensor_scalar(mk[:], w_sb[:, XD:EH],
                                        0.25, 0.25, OP.is_gt, OP.subtract)
                nc.gpsimd.tensor_tensor(w_sb[:, COS0 + XD:E2],
                                        w_sb[:, XD:EH], mk[:],
                                        OP.subtract)

                if (i % 2 == 0) and not single:
                    continue
                if single:
                    nc.scalar.activation(P2[:, 0:E2], w2[:, 0:E2], AF.Sin,
                                         scale=float(TWOPI))
                    group = (i,)
                else:
                    nc.scalar.activation(P2[:], w2[:], AF.Sin,
                                         scale=float(TWOPI))
                    group = (i - 1, i)

                for j in group:
                  P = P2[:, (0 if (j == i and single) or j % 2 == 0 else E2):][:, 0:E2]
                  if True:
                    qt = qall_sb[:, j:j + 1]
                    # qEz [128 atoms, 128 cols]: 0-56 q*EzRe(c=-28..28),
                    # 57-63 zero, 64-120 q*EzIm, 121-127 zero
                    qez = qpool.tile([128, 128], f32r, tag="qez")
                    nc.gpsimd.memset(qez[:, 57:64].bitcast(f32), 0.0)
                    nc.gpsimd.memset(qez[:, 121:128].bitcast(f32), 0.0)
                    cosz = P[:, COSZ0:COSZ0 + A]
                    sinz = P[:, SINZ0:SINZ0 + A]
                    nc.gpsimd.tensor_scalar(qez[:, 0:29], cosz[:, ::-1],
                                            qt[:], None, OP.mult)
                    nc.gpsimd.tensor_scalar(qez[:, 29:57], cosz[:, 1:29],
                                            qt[:], None, OP.mult)
                    qneg = qpool.tile([128, 1], f32, tag="qneg")
                    nc.gpsimd.tensor_scalar(qneg[:], qt[:], -1.0, None,
                                            OP.mult)
                    nc.gpsimd.tensor_scalar(qez[:, 64:93], sinz[:, ::-1],
                                            qneg[:], None, OP.mult)
                    nc.gpsimd.tensor_scalar(qez[:, 93:121], sinz[:, 1:29],
                                            qt[:], None, OP.mult)

                    first = j == 0
                    last = j == NTILES - 1
                    for St, s0, p0, ln in (
                            (Ssin, S_SIN, 0, S_CUT),
                            (Ssin, S_SIN + S_CUT, S_CUT, E_XY - S_CUT),
                            (Scos, S_SIN, COS0, S_CUT),
                            (Scos, S_SIN + S_CUT, COS0 + S_CUT,
                             E_XY - S_CUT)):
                        nc.tensor.matmul(St[:, s0:s0 + ln], qez[:],
                                         P[:, p0:p0 + ln],
                                         start=first, stop=last,
                                         skip_group_check=True)

            # ---- epilogue ----
            # t1 = S_cosRe - S_sinIm, t2 = S_sinRe + S_cosIm, both * sqrt(w),
            # squared + row-accumulated; host sums the 128 partials.
            # DVE handles the t1 chain, Pool the t2 chain, in parallel.
            sqw_sb = cpool.tile([C, E_XY], f32)
            nc.sync.dma_start(sqw_sb[:], sqw[:])

            qsq = epool.tile([128, NTILES], f32, tag="qsq")
            nc.scalar.activation(qsq[:], qall_sb[:], AF.Square,
                                 accum_out=acc[:, 2:3])

            # S (PSUM) readers limited to ACT + DVE (GPSIMD cannot touch
            # PSUM); Pool applies the sqrt(w) scaling from SBUF, ACT
            # squares+accumulates
            simA = epool.tile([C, E_XY], f32, tag="simA")
            nc.scalar.activation(simA[:], Ssin[64:64 + C, S_SIN:S_SIN + E_XY],
                                 AF.Identity)
            simB = epool.tile([C, E_XY], f32, tag="simB")
            nc.scalar.activation(simB[:], Scos[64:64 + C, S_SIN:S_SIN + E_XY],
                                 AF.Identity)
            t1 = epool.tile([C, E_XY], f32, tag="t1")
            nc.vector.tensor_tensor(t1[:], Scos[0:C, S_SIN:S_SIN + E_XY],
                                    simA[:], OP.subtract)
            t2 = epool.tile([C, E_XY], f32, tag="t2")
            nc.vector.tensor_tensor(t2[:], Ssin[0:C, S_SIN:S_SIN + E_XY],
                                    simB[:], OP.add)
            m1 = epool.tile([C, E_XY], f32, tag="m1")
            nc.gpsimd.tensor_tensor(m1[:], t1[:], sqw_sb[:], OP.mult)
            m2 = epool.tile([C, E_XY], f32, tag="m2")
            nc.gpsimd.tensor_tensor(m2[:], t2[:], sqw_sb[:], OP.mult)
            h1 = epool.tile([C, E_XY], f32, tag="h1")
            nc.scalar.activation(h1[:], m1[:], AF.Square,
                                 accum_out=acc[0:C, 0:1])
            h2 = epool.tile([C, E_XY], f32, tag="h2")
            nc.scalar.activation(h2[:], m2[:], AF.Square,
                                 accum_out=acc[0:C, 1:2])
            nc.sync.dma_start(res[:], acc[:])

    nc.compile()
    return nc


def _make_in_maps(positions, q, cell):
    positions = np.asarray(positions, np.float32)
    q = np.asarray(q, np.float32)
    cell = np.asarray(cell, np.float32)

    in_maps = []
    for core in range(8):
        f, h = core // 2, core % 2
        box = np.diagonal(cell[f])
        avec, twohot, sqwt = _build_tables(h, box)
        pT = np.zeros((3, NPAD), np.float32)
        pT[:, :N] = positions[f * N:(f + 1) * N].T
        qc = np.zeros((NPAD, 1), np.float32)
        qc[:N, 0] = q[f * N:(f + 1) * N, 0]
        in_maps.append({
            "posT": np.ascontiguousarray(pT), "qcol": qc, "avec": avec,
            "twohot": twohot, "sqw": sqwt,
        })
    return in_maps


def kernel(positions, q, cell, batch, _want_trace=False):
    global _compiled
    from concourse.bass_utils import run_bass_kernel_spmd

    if _compiled is None:
        _compiled = _build_module()
    nc = _compiled

    in_maps = _make_in_maps(positions, q, cell)
    r = run_bass_kernel_spmd(nc, in_maps, core_ids=list(range(8)),
                             trace=_want_trace)
    pots = np.zeros(B, np.float32)
    for f in range(B):
        a = r.results[2 * f]["res"]
        b = r.results[2 * f + 1]["res"]
        ws2 = (a[:, 0].sum() + a[:57, 1].sum()
               + b[:, 0].sum() + b[:57, 1].sum())
        pots[f] = ws2 - np.float32(SELF_CONST) * a[:, 2].sum()
    kernel._last_results = r
    return pots
